# revision 1
# baseline (speedup 1.0000x reference)
"""Distributed CFGCN propagate_embedding kernel for 8 TRN2 NeuronCores.

Strategy (1D graph partitioning):
  - Nodes are split into 8 contiguous slices of N/8; core d owns slice d.
  - Edges are partitioned by destination core; each core owns the
    segment-sum for its destination nodes.
  - Each core keeps a full replicated table of scaled node features
    (x * sqrt_degree) in DRAM, rebuilt once per layer with an AllGather
    (each rank contributes its owned slice).
  - Per layer, each core gathers the source rows for its ~E/8 edges with
    indirect DMA and accumulates them in SBUF.  The edge schedule is
    arranged on the host into "rounds": the nodes owned by a core are
    sorted by in-degree ("slots"), so round r gathers the r-th incoming
    edge of every node that has one — a dense, slot-aligned [128 x cols]
    rectangle per round, accumulated via compute_op=add (one indirect
    DMA per 128-slot column).  Nodes whose degree is < r are padded with
    an index pointing at a zero row of the table, so no masking is
    needed anywhere.
  - mean over layers uses sum_l x_l = sqrt_degree * sum_l agg_l, so the
    device only accumulates raw aggregate sums; the host applies the
    final sqrt_degree scale, adds the ego embedding and divides by 4.

The table layout is slot-ordered per rank block (not global node order):
host-side index translation bakes every permutation into the gather
indices, so the device never permutes anything.
"""

import numpy as np

N_CORES = 8
P = 128  # SBUF partitions


def _build_schedule(emb, sqrt_degree, src, dst):
    """Host-side scheduling: degree-sorted slots, rounds, gather indices.

    Returns everything needed to build the bass program and the in_maps.
    """
    n_nodes, d_model = emb.shape
    npc = n_nodes // N_CORES            # nodes per core
    w = -(-npc // P)                    # slot columns per partition
    if w * P == npc:
        w += 1                          # force pad slots to exist (zero rows)
    slots = w * P                       # padded slots per core
    zrow = npc                          # core 0's first pad slot: always zero

    deg = np.bincount(dst, minlength=n_nodes)

    # per-core degree-sorted orders and global node -> table-row map
    orders = []
    table_row_of = np.empty(n_nodes, dtype=np.int64)
    for d in range(N_CORES):
        lo = d * npc
        degl = deg[lo:lo + npc]
        order = np.argsort(-degl, kind="stable")       # slotrank -> local node
        orders.append(order)
        rank_of = np.empty(npc, dtype=np.int64)
        rank_of[order] = np.arange(npc)
        table_row_of[lo:lo + npc] = d * slots + rank_of

    # per-core edge matrices M[r, slotrank] = table row of src (or zrow)
    per_core_M = []
    per_core_crs = []
    r_max = 0
    for d in range(N_CORES):
        lo = d * npc
        mask = (dst >= lo) & (dst < lo + npc)
        es = src[mask].astype(np.int64)
        ed = dst[mask].astype(np.int64) - lo
        rank_of = np.empty(npc, dtype=np.int64)
        rank_of[orders[d]] = np.arange(npc)
        er = rank_of[ed]                                # slotrank per edge
        o = np.argsort(er, kind="stable")
        er_s = er[o]
        es_s = es[o]
        # occurrence index of each edge within its slotrank group
        starts = np.searchsorted(er_s, np.arange(npc), side="left")
        occ = np.arange(len(er_s)) - starts[er_s]
        rd = int(occ.max()) + 1 if len(occ) else 1
        m = np.full((rd, slots), zrow, dtype=np.int64)
        m[occ, er_s] = table_row_of[es_s]
        per_core_M.append(m)
        # c_r = number of slots with degree > r  (sorted -> prefix)
        degl_sorted = deg[lo:lo + npc][orders[d]]
        crs = np.array([(degl_sorted > r).sum() for r in range(rd)], dtype=np.int64)
        per_core_crs.append(crs)
        r_max = max(r_max, rd)

    # shared round structure: cols_r = max over cores of ceil(c_r / P)
    cols = np.zeros(r_max, dtype=np.int64)
    for d in range(N_CORES):
        crs = per_core_crs[d]
        for r in range(len(crs)):
            cols[r] = max(cols[r], -(-crs[r] // P))
    cols[0] = w                                        # round 0 covers all slots
    offsets = np.zeros(r_max + 1, dtype=np.int64)
    offsets[1:] = np.cumsum(cols)
    totcol = int(offsets[-1])

    # per-core gather index tiles [P, totcol]
    idx_tiles = []
    for d in range(N_CORES):
        m = per_core_M[d]
        idx = np.full((P, totcol), zrow, dtype=np.int32)
        for r in range(r_max):
            ncol = int(cols[r])
            if r < m.shape[0]:
                # slot s = c*P + p  ->  idx[p, off+c]
                blk = m[r, :ncol * P].reshape(ncol, P).T
                idx[:, offsets[r]:offsets[r] + ncol] = blk.astype(np.int32)
        idx_tiles.append(idx)

    # per-core slot-ordered host data
    sd = sqrt_degree.reshape(-1)
    scaled0 = []
    sd2_tiles = []
    for d in range(N_CORES):
        lo = d * npc
        order = orders[d]
        s0 = np.zeros((slots, d_model), dtype=np.float32)
        s0[:npc] = (emb[lo:lo + npc] * sqrt_degree[lo:lo + npc])[order]
        scaled0.append(s0)
        s2 = np.zeros(slots, dtype=np.float32)
        s2[:npc] = (sd[lo:lo + npc] ** 2)[order]
        # slot s = c*P + p -> sbuf position (p, c); expand per feature
        s2_pc = s2.reshape(w, P).T                      # [P, w]
        sd2_tiles.append(np.repeat(s2_pc, d_model, axis=1).astype(np.float32))

    sched = {
        "n_nodes": n_nodes, "d": d_model, "npc": npc, "w": w,
        "slots": slots, "zrow": zrow, "r_max": r_max,
        "cols": cols, "offsets": offsets, "totcol": totcol,
        "orders": orders,
        "idx_tiles": idx_tiles, "scaled0": scaled0, "sd2_tiles": sd2_tiles,
    }
    return sched


def _build_program(sched, n_layers):
    from concourse import bacc, bass, mybir, tile

    f32 = mybir.dt.float32
    i32 = mybir.dt.int32
    D = sched["d"]
    W = sched["w"]
    SLOTS = sched["slots"]
    TOTCOL = sched["totcol"]
    cols = sched["cols"]
    offsets = sched["offsets"]
    r_max = sched["r_max"]
    TROWS = N_CORES * SLOTS

    nc = bacc.Bacc("TRN2", target_bir_lowering=False, debug=False,
                   num_devices=N_CORES)
    scaled0_in = nc.dram_tensor("scaled0", [SLOTS, D], f32, kind="ExternalInput")
    idx_in = nc.dram_tensor("idx", [P, TOTCOL], i32, kind="ExternalInput")
    sd2_in = nc.dram_tensor("sd2", [P, W * D], f32, kind="ExternalInput")
    out_d = nc.dram_tensor("out", [P, W * D], f32, kind="ExternalOutput")

    with tile.TileContext(nc) as tc:
        with tc.tile_pool(name="dram", bufs=1, space="DRAM") as dpool, \
             tc.tile_pool(name="sb", bufs=1) as sb:
            tables = [
                dpool.tile([TROWS, D], f32, addr_space="Shared", name=f"tbl{l}")
                for l in range(n_layers)
            ]
            S_in = dpool.tile([SLOTS, D], f32, name="agin")

            idx_sb = sb.tile([P, TOTCOL], i32, name="idx_sb")
            sd2_sb = sb.tile([P, W * D], f32, name="sd2_sb")
            acc = sb.tile([P, W * D], f32, name="acc")
            mean = sb.tile([P, W * D], f32, name="mean")

            nc.sync.dma_start(out=idx_sb[:], in_=idx_in[:])
            nc.sync.dma_start(out=sd2_sb[:], in_=sd2_in[:])
            # initial table: AllGather of host-computed scaled0
            nc.gpsimd.dma_start(out=S_in[:], in_=scaled0_in[:])
            nc.gpsimd.collective_compute(
                "AllGather", mybir.AluOpType.bypass,
                replica_groups=[list(range(N_CORES))],
                ins=[S_in[:]], outs=[tables[0][:, :]])

            for layer in range(1, n_layers + 1):
                T = tables[layer - 1]
                for r in range(r_max):
                    for c in range(int(cols[r])):
                        col = int(offsets[r]) + c
                        op = (mybir.AluOpType.bypass if r == 0
                              else mybir.AluOpType.add)
                        nc.gpsimd.indirect_dma_start(
                            out=acc[:, c * D:(c + 1) * D],
                            out_offset=None,
                            in_=T[:, :],
                            in_offset=bass.IndirectOffsetOnAxis(
                                ap=idx_sb[:, col:col + 1], axis=0),
                            compute_op=op)
                if layer == 1:
                    nc.vector.tensor_copy(out=mean[:], in_=acc[:])
                else:
                    nc.vector.tensor_add(out=mean[:], in0=mean[:], in1=acc[:])
                if layer < n_layers:
                    nc.vector.tensor_tensor(
                        out=acc[:], in0=acc[:], in1=sd2_sb[:],
                        op=mybir.AluOpType.mult)
                    # flush slot s=c*P+p (sbuf (p, c)) -> S_in row s
                    nc.sync.dma_start(
                        out=S_in[:].rearrange("(c p) f -> p c f", p=P),
                        in_=acc[:].rearrange("p (c f) -> p c f", f=D))
                    nc.gpsimd.collective_compute(
                        "AllGather", mybir.AluOpType.bypass,
                        replica_groups=[list(range(N_CORES))],
                        ins=[S_in[:]], outs=[tables[layer][:, :]])
            nc.sync.dma_start(out=out_d[:], in_=mean[:])
    nc.compile()
    return nc


def kernel(**inputs):
    emb = np.ascontiguousarray(np.asarray(inputs["emb"], dtype=np.float32))
    sqrt_degree = np.ascontiguousarray(
        np.asarray(inputs["sqrt_degree"], dtype=np.float32))
    src = np.asarray(inputs["src"], dtype=np.int32)
    dst = np.asarray(inputs["dst"], dtype=np.int32)
    n_layers = 3

    sched = _build_schedule(emb, sqrt_degree, src, dst)
    nc = _build_program(sched, n_layers)

    from concourse.bass_utils import run_bass_kernel_spmd
    in_maps = [
        {"scaled0": sched["scaled0"][d],
         "idx": sched["idx_tiles"][d],
         "sd2": sched["sd2_tiles"][d]}
        for d in range(N_CORES)
    ]
    res = run_bass_kernel_spmd(nc, in_maps, list(range(N_CORES)))

    n_nodes, d_model = emb.shape
    npc = sched["npc"]
    W = sched["w"]
    out = np.empty((n_nodes, d_model), dtype=np.float32)
    sd = sqrt_degree.reshape(-1)
    for d in range(N_CORES):
        lo = d * npc
        dev = res.results[d]["out"]                     # [P, W*D] slot layout
        agg = dev.reshape(P, W, d_model).transpose(1, 0, 2).reshape(-1, d_model)
        order = sched["orders"][d]
        loc = np.empty((npc, d_model), dtype=np.float32)
        loc[order] = agg[:npc]
        out[lo:lo + npc] = (emb[lo:lo + npc]
                            + sd[lo:lo + npc, None] * loc) / (n_layers + 1)
    return out

